# revision 62
# baseline (speedup 1.0000x reference)
"""Trainium2 Bass kernel for nn_ByteSequenceEmbedder (fp8 DoubleRow version).

Model (per sequence, 8 sequences data-parallel over 8 NeuronCores):
  x  = tok_emb[tokens] + bpe*E[4] + word*E[3]                 [T=4096, 64]
  x  = relu(conv3(x, W0) + b0); 2x highway(512)               [T, 512]
  x  = relu(conv3(x, W1) + b1 + x); 2x highway(512)           [T, 512]
  x  = per-word segment max (ragged, sorted seg_ids, W=1024)  [W, 512]
  out= x @ Pw + Pb                                            [W, 512]

Precision strategy (validated in numpy sim, rel_err 4.5e-3 == bf16 level):
 - highway gate gemms: plain fp8e4 DoubleRow (sigmoid attenuates noise)
 - highway h gemms + conv1: hi/lo-split fp8e4 DoubleRow:
     W*WS ~ A + B (A=e4(W*WS), B=e4(W*WS-A)); y*XS ~ u + v
     W@y * WS*XS ~ A@u + A@v + B@u  (3 products, B@v dropped)
   DoubleRow packs 2 (lhsT,rhs) K<=128 tile pairs per instr at 0.5
   cyc/row -> 4x bf16 throughput per product.
 - conv0 (K=64): bf16 with taps 0+1 K-packed into one K=128 matmul
 - proj, transposes: bf16. Element-wise stream (y, g, h) stays bf16.
 - XS=128, WS=64 (powers of 2; PSUM evicted with scale 2^-13)

Engine split: ACT: psum evictions (g sigmoid / h relu / conv relu, fp8 u
from conv psums, fp8 u' copies of highway outputs). DVE: highway combine
(3x tensor_tensor bf16), all v-residual stt ops, transpose-psum/proj-psum
evacuation copies, segment-max tree. Pool: SWDGE gathers only (walrus
rejects ALU ops on Pool). PE: matmuls + transposes. PSUM: 4x[128,1024]
slots; u'/v' emission staggered one superchunk behind the evictions so the
in-order ACT/DVE queues never stall the next layer's PE stream.
"""

import functools
import os
import sys

import numpy as np

for _p in ("/opt/trn_rl_repo", "/root/.axon_site/_ro/trn_rl_repo"):
    if os.path.isdir(_p) and _p not in sys.path:
        sys.path.append(_p)

import ml_dtypes  # noqa: E402

from contextlib import ExitStack  # noqa: E402

from concourse import bacc, bass, mybir, tile  # noqa: E402
from concourse import library_config  # noqa: E402
from concourse.bass_utils import run_bass_kernel_spmd  # noqa: E402

B, T, W = 8, 4096, 1024
DB, DW = 64, 512
NH = 2
VOCAB = 264
BPE_MARK, WORD_MARK = 4, 3
SC = 1024          # tokens per super-chunk (psum tile free size)
NSC = T // SC
NMM = 512          # bf16 matmul moving columns
NDR = 256          # DoubleRow matmul output columns (rhs moving = 512)
MCH = DW // 128    # output-feature chunks
KCH = DW // 128    # contraction chunks
NCORES = 8
CVOCAB = 4 * VOCAB  # combined (tok, bpe, word) vocabulary

XS = 128.0          # activation fp8 scale (2^7)
WS = 64.0           # weight fp8 scale (2^6)
EVS = 1.0 / (XS * WS)   # psum eviction scale 2^-13

BF16 = mybir.dt.bfloat16
F32 = mybir.dt.float32
E4 = mybir.dt.float8e4
I16 = mybir.dt.int16
AF = mybir.ActivationFunctionType
OP = mybir.AluOpType
DR = mybir.MatmulPerfMode.DoubleRow

bf16_np = ml_dtypes.bfloat16
e4_np = ml_dtypes.float8_e4m3

# bias column layout in bias_sb [128, 48] f32
COL_C0, COL_C0X = 0, 4        # conv0_b, conv0_b*XS
COL_C1, COL_C1X = 8, 12       # conv1_b, conv1_b*XS


def _col_g(block, l):
    return 16 + (block * NH + l) * 8


def _col_h(block, l):
    return 16 + (block * NH + l) * 8 + 4


def build_program(ntaps: int) -> bass.Bass:
    nc = bacc.Bacc("TRN2", target_bir_lowering=False, debug=False)

    def din(name, shape, dtype):
        return nc.dram_tensor(name, list(shape), dtype, kind="ExternalInput")

    emb_d = din("emb_comb", (CVOCAB, 128), BF16)
    tokidx_d = din("tok_idx", (128, T // 16), I16)
    w0p_d = din("w0p", (128, DW), BF16)          # taps 0,1 K-stacked
    w0t2_d = din("w0t2", (DB, DW), BF16)         # tap 2
    ag_d = din("ag", (128, 2, NH, KCH, DW), E4)  # plain gate weights [block]
    wh_d = din("wh", (128, 2, NH, 2, KCH, DW), E4)  # h weights [block][B;A]
    c1_d = din("c1", (128, 2, 3, KCH, DW), E4)      # conv1 [B;A][tap]
    projw_d = din("projw", (128, KCH, DW), BF16)
    projb_d = din("projb", (1, DW), BF16)
    bias_d = din("biases", (128, 48), F32)
    ident_d = din("ident", (128, 128), BF16)
    gidx_d = din("gidx", (128, 8 * ntaps * 8), I16)
    out_d = nc.dram_tensor("out", [W, DW], F32, kind="ExternalOutput")
    y1t_d = nc.dram_tensor("y1t", [T, DW], BF16, kind="Internal")

    with tile.TileContext(nc) as tc, ExitStack() as ctx:
        const = ctx.enter_context(tc.tile_pool(name="const", bufs=1))
        ps = ctx.enter_context(tc.tile_pool(name="psp", bufs=4, space="PSUM"))
        gp = ctx.enter_context(tc.tile_pool(name="gpool", bufs=8))
        hp = ctx.enter_context(tc.tile_pool(name="hpool", bufs=4))
        dp = ctx.enter_context(tc.tile_pool(name="dpool", bufs=3))
        uvp = ctx.enter_context(tc.tile_pool(name="uvpool", bufs=2))
        tp = ctx.enter_context(tc.tile_pool(name="tpool", bufs=4))
        gat = ctx.enter_context(tc.tile_pool(name="gat", bufs=2))
        obp = ctx.enter_context(tc.tile_pool(name="obp", bufs=2))

        nc.gpsimd.load_library(library_config.mlp)

        def load(dram_t, shape, dtype, name):
            t = const.tile(shape, dtype, name=name)
            nc.sync.dma_start(out=t[:], in_=dram_t[:])
            return t

        # conv0 dependencies first (HWDGE is FIFO per engine)
        tokidx_sb = load(tokidx_d, [128, T // 16], I16, "tokidx_sb")
        w0p_sb = load(w0p_d, [128, DW], BF16, "w0p_sb")
        w0t2_sb = load(w0t2_d, [DB, DW], BF16, "w0t2_sb")
        bias_sb = load(bias_d, [128, 48], F32, "bias_sb")

        # ---- embedding gather: xg[p, t] = emb_comb[cidx[t], p] ----
        # table cols 64:128 duplicate 0:64 so xg[64:128,:] == x too.
        # Shares the gather pool: dead after conv0.
        xg = gat.tile([128, T], BF16, tag="tap", name="xg")
        EC = T // 4
        for r in range(4):
            nc.gpsimd.dma_gather(
                out_ap=xg[:, r * EC:(r + 1) * EC].rearrange(
                    "p (c n) -> p c n", c=1),
                in_ap=emb_d[:],
                idxs_ap=tokidx_sb[:, r * (EC // 16):(r + 1) * (EC // 16)],
                num_idxs=EC,
                num_idxs_reg=EC,
                elem_size=128,
                transpose=True,
                single_packet=False,
            )

        ag_sb = load(ag_d, [128, 2, NH, KCH, DW], E4, "ag_sb")
        wh_sb = load(wh_d, [128, 2, NH, 2, KCH, DW], E4, "wh_sb")
        c1_sb = load(c1_d, [128, 2, 3, KCH, DW], E4, "c1_sb")
        projw_sb = load(projw_d, [128, KCH, DW], BF16, "projw_sb")
        projb_sb = load(projb_d, [1, DW], BF16, "projb_sb")
        gidx_sb = load(gidx_d, [128, 8 * ntaps * 8], I16, "gidx_sb")
        ident_sb = load(ident_d, [128, 128], BF16, "ident_sb")
        ones_sb = const.tile([1, 128], BF16, name="ones_sb")
        nc.vector.memset(ones_sb[:], 1.0)

        # xg2: partitions 0:64 = x[t-1], 64:128 = x[t]  (conv0 tap packing)
        # Shares the uv pool: dead before uv1 is written.
        xg2 = uvp.tile([128, T], BF16, tag="uv", name="xg2")
        nc.vector.memset(xg2[0:DB, 0:1], 0.0)
        for r in range(4):
            c0, c1 = r * EC, (r + 1) * EC
            nc.vector.tensor_copy(
                out=xg2[0:DB, c0 + 1:c1 + 1] if r < 3
                else xg2[0:DB, c0 + 1:T],
                in_=xg[0:DB, c0:c1] if r < 3 else xg[0:DB, c0:T - 1])
            nc.vector.tensor_copy(out=xg2[DB:128, c0:c1], in_=xg[DB:128, c0:c1])

        y_tiles = [const.tile([128, T], BF16, name=f"y_{m}")
                   for m in range(MCH)]

        def uv_tile(name):
            return uvp.tile([128, 2 * KCH, T], E4, tag="uv", name=name)

        def conv_v_pass(uv_out, sc):
            for m in range(MCH):
                cols = slice(sc * SC, sc * SC + SC)
                nc.vector.scalar_tensor_tensor(
                    out=uv_out[:, KCH + m, cols], in0=y_tiles[m][:, cols],
                    scalar=XS, in1=uv_out[:, m, cols],
                    op0=OP.mult, op1=OP.subtract,
                )

        # ---------- conv0 (bf16, 2-tap packed), evict y0 + u0 ----------
        uv0 = uv_tile("uv0")

        def conv0_gen():
            for sc in range(NSC):
                base = sc * SC
                for m in range(MCH):
                    mc = slice(m * 128, (m + 1) * 128)
                    pc = ps.tile([128, SC], F32, tag="ps", name="pc")
                    for n in range(SC // NMM):
                        c0 = n * NMM
                        t0 = base + c0
                        # taps 0+1 via xg2 (K=128), full region
                        nc.tensor.matmul(
                            out=pc[:, c0:c0 + NMM],
                            lhsT=w0p_sb[:, mc],
                            rhs=xg2[:, t0:t0 + NMM],
                            start=True, stop=False,
                        )
                        # tap 2 from xg[0:64] shifted +1, clipped at T
                        lo = t0 + 1
                        ln = min(NMM, T - lo)
                        nc.tensor.matmul(
                            out=pc[:, c0:c0 + ln],
                            lhsT=w0t2_sb[:, mc],
                            rhs=xg[0:DB, lo:lo + ln],
                            start=False, stop=True,
                        )
                    cols = slice(base, base + SC)
                    nc.scalar.activation(
                        out=y_tiles[m][:, cols], in_=pc[:], func=AF.Relu,
                        bias=bias_sb[:, COL_C0 + m:COL_C0 + m + 1],
                    )
                    nc.scalar.activation(
                        out=uv0[:, m, cols], in_=pc[:], func=AF.Relu,
                        bias=bias_sb[:, COL_C0X + m:COL_C0X + m + 1],
                        scale=XS,
                    )
                if sc >= 1:
                    conv_v_pass(uv0, sc - 1)
                yield
            conv_v_pass(uv0, NSC - 1)
            yield

        # ---------- highway layer helper ----------
        def uv_pass(uv_out, sc):
            """fp8 hi/lo of y for superchunk sc: u on ACT (except one
            feature chunk on DVE for engine balance), v on DVE."""
            for m in range(MCH):
                cols = slice(sc * SC, sc * SC + SC)
                yap = y_tiles[m][:, cols]
                if m == MCH - 1:
                    nc.vector.tensor_scalar(
                        out=uv_out[:, m, cols], in0=yap,
                        scalar1=XS, scalar2=None, op0=OP.mult,
                    )
                else:
                    nc.scalar.activation(
                        out=uv_out[:, m, cols], in_=yap,
                        func=AF.Copy, scale=XS,
                    )
                nc.vector.scalar_tensor_tensor(
                    out=uv_out[:, KCH + m, cols], in0=yap,
                    scalar=XS, in1=uv_out[:, m, cols],
                    op0=OP.mult, op1=OP.subtract,
                )

        def hw_layer(block, l, uv_in, uv_out, last, tail_fn=None):
            """One highway layer: consume uv_in, update y_tiles in place,
            and (unless last) produce uv_out = fp8 hi/lo of the new y.

            Emission order staggers the two superchunks so that sc0's
            u/v (ACT+DVE) are queued behind sc1's g evictions: ACT keeps
            working while DVE finishes sc0 combines, and the next layer's
            PE work only waits on sc0's u/v."""
            colg, colh = _col_g(block, l), _col_h(block, l)

            def g_phase(sc):
                base = sc * SC
                g_tiles = []
                for m in range(MCH):
                    mc = slice(m * 128, (m + 1) * 128)
                    pg = ps.tile([128, SC], F32, tag="ps", name="pg")
                    for n in range(SC // NDR):
                        c0 = n * NDR
                        for kp in range(KCH // 2):
                            nc.tensor.matmul(
                                out=pg[:, c0:c0 + NDR],
                                lhsT=ag_sb[:, block, l, 2 * kp:2 * kp + 2, mc],
                                rhs=uv_in[:, 2 * kp:2 * kp + 2,
                                          base + c0:base + c0 + NDR],
                                start=(kp == 0), stop=(kp == KCH // 2 - 1),
                                perf_mode=DR,
                            )
                    g = gp.tile([128, SC], BF16, tag="g", name="g")
                    nc.scalar.activation(
                        out=g[:], in_=pg[:], func=AF.Sigmoid,
                        bias=bias_sb[:, colg + m:colg + m + 1], scale=EVS,
                    )
                    g_tiles.append(g)
                return g_tiles

            def h_combine_phase(sc, g_tiles):
                base = sc * SC
                for m in range(MCH):
                    mc = slice(m * 128, (m + 1) * 128)
                    ph = ps.tile([128, SC], F32, tag="ps", name="ph")
                    for n in range(SC // NDR):
                        c0, c1 = base + n * NDR, base + (n + 1) * NDR
                        for kp in range(KCH // 2):  # main: A@u chunk pairs
                            nc.tensor.matmul(
                                out=ph[:, n * NDR:(n + 1) * NDR],
                                lhsT=wh_sb[:, block, l, 1,
                                           2 * kp:2 * kp + 2, mc],
                                rhs=uv_in[:, 2 * kp:2 * kp + 2, c0:c1],
                                start=(kp == 0), stop=False,
                                perf_mode=DR,
                            )
                        for k in range(KCH):        # cross: B@u + A@v
                            nc.tensor.matmul(
                                out=ph[:, n * NDR:(n + 1) * NDR],
                                lhsT=wh_sb[:, block, l, :, k, mc],
                                rhs=uv_in[:, k:k + KCH + 1:KCH, c0:c1],
                                start=False, stop=(k == KCH - 1),
                                perf_mode=DR,
                            )
                    h = hp.tile([128, SC], BF16, tag="h", name="h")
                    nc.scalar.activation(
                        out=h[:], in_=ph[:], func=AF.Relu,
                        bias=bias_sb[:, colh + m:colh + m + 1], scale=EVS,
                    )
                    cols = slice(base, base + SC)
                    yap = y_tiles[m][:, cols]
                    d = dp.tile([128, SC], BF16, tag="d", name="d")
                    nc.vector.tensor_tensor(
                        out=d[:], in0=h[:], in1=yap, op=OP.subtract)
                    nc.vector.tensor_tensor(
                        out=d[:], in0=g_tiles[m][:], in1=d[:], op=OP.mult)
                    nc.vector.tensor_tensor(
                        out=yap, in0=yap, in1=d[:], op=OP.add)

            gs = g_phase(0)
            h_combine_phase(0, gs)
            if tail_fn is not None:
                tail_fn(0)
            yield
            for sc in range(1, NSC):
                gs = g_phase(sc)
                if not last:
                    uv_pass(uv_out, sc - 1)
                h_combine_phase(sc, gs)
                if tail_fn is not None:
                    tail_fn(sc)
                yield
            if not last:
                uv_pass(uv_out, NSC - 1)
            yield

        # ---------- conv1 (hi/lo fp8 DR, residual folded), evict y1+u1 ----
        uv1 = uv_tile("uv1")
        uv2 = uv_tile("uv2")
        uv3 = uv_tile("uv3")
        uv4 = uv_tile("uv4")

        def conv1_gen():
            for sc in range(NSC):
                base = sc * SC
                for m in range(MCH):
                    mc = slice(m * 128, (m + 1) * 128)
                    pc = ps.tile([128, SC], F32, tag="ps", name="pc1")
                    for n in range(SC // NDR):
                        c0 = n * NDR
                        first = True
                        for t in (1, 0, 2):
                            lo = base + c0 + (t - 1)
                            ln = NDR
                            o0 = c0
                            if lo < 0:
                                lo, ln, o0 = 0, NDR - 1, c0 + 1
                            elif lo + ln > T:
                                ln = T - lo
                            for kp in range(KCH // 2):  # main A@u
                                nc.tensor.matmul(
                                    out=pc[:, o0:o0 + ln],
                                    lhsT=c1_sb[:, 1, t,
                                               2 * kp:2 * kp + 2, mc],
                                    rhs=uv2[:, 2 * kp:2 * kp + 2,
                                            lo:lo + ln],
                                    start=first, stop=False,
                                    perf_mode=DR,
                                )
                                first = False
                            for k in range(KCH):        # cross B@u + A@v
                                nc.tensor.matmul(
                                    out=pc[:, o0:o0 + ln],
                                    lhsT=c1_sb[:, :, t, k, mc],
                                    rhs=uv2[:, k:k + KCH + 1:KCH,
                                            lo:lo + ln],
                                    start=False,
                                    stop=(t == 2 and k == KCH - 1),
                                    perf_mode=DR,
                                )
                    cols = slice(base, base + SC)
                    nc.scalar.activation(
                        out=y_tiles[m][:, cols], in_=pc[:], func=AF.Relu,
                        bias=bias_sb[:, COL_C1 + m:COL_C1 + m + 1],
                        scale=EVS,
                    )
                    nc.scalar.activation(
                        out=uv3[:, m, cols], in_=pc[:], func=AF.Relu,
                        bias=bias_sb[:, COL_C1X + m:COL_C1X + m + 1],
                        scale=EVS * XS,
                    )
                if sc >= 1:
                    conv_v_pass(uv3, sc - 1)
                yield
            conv_v_pass(uv3, NSC - 1)
            yield
        # ---- transpose y -> token-major, bounce to DRAM ----
        # batched: 8 transposes -> one [128,1024] psum tile -> one DVE copy
        # -> one 256-row DMA; interleaved per superchunk into the last
        # highway layer so y1t is mostly written (and gathers fire) before
        # the body ends.
        def transpose_chunk(sc):
            for gi in range(sc * (SC // 256), (sc + 1) * (SC // 256)):
                pt = ps.tile([128, 1024], BF16, tag="ps", name="pt")
                for i in range(2):
                    col = gi * 256 + i * 128
                    for m in range(MCH):
                        nc.tensor.transpose(
                            out=pt[:, i * 512 + m * 128:
                                   i * 512 + (m + 1) * 128],
                            in_=y_tiles[m][:, col:col + 128],
                            identity=ident_sb[:],
                        )
                st = tp.tile([128, 1024], BF16, tag="y1t", name="st")
                nc.vector.tensor_copy(out=st[:], in_=pt[:])
                row0 = gi * 256
                nc.sync.dma_start(
                    out=y1t_d[row0:row0 + 256, :].rearrange(
                        "(i p) f -> p i f", i=2),
                    in_=st[:].rearrange("p (i f) -> p i f", i=2),
                )

        # ---------- wavefront driver ----------
        # Each phase is a generator with NSC+1 steps. A consumer phase's
        # step s needs its producer's uv slice for superchunk s, which the
        # producer finishes `off` steps later (staggered v/u passes; conv1
        # additionally needs one halo column of the next superchunk).
        # Interleaved emission puts ACT/DVE work of highway layers inside
        # conv1's PE-heavy window and vice versa.
        # Highway layers are split into compute (matmuls+evictions+combine)
        # and uvout (fp8 u/v of the updated y) generators so compute can
        # overlap a predecessor at depth 2 while uv WRITES wait until the
        # recycled uv buffer's previous readers have been emitted (the uv
        # pool double-buffers; writer-before-reader emission wedges the
        # in-order ACT queue into a deadlock). Gate: consumer step s waits
        # until done[pred] >= min(s + off, total[pred]).
        chain = [
            (conv0_gen(), 5, []),                                  # 0
            (hw_layer(0, 0, uv0, uv1, False), 5, [(0, 3)]),        # 1
            (hw_layer(0, 1, uv1, uv2, False), 5, [(1, 3)]),        # 2
            (conv1_gen(), 5, [(2, 4)]),                            # 3
            (hw_layer(1, 0, uv3, uv4, False), 5, [(3, 3)]),        # 4
            (hw_layer(1, 1, uv4, None, True,
                      tail_fn=transpose_chunk), 5, [(4, 2)]),      # 5
        ]
        done = [0] * len(chain)
        while any(done[i] < chain[i][1] for i in range(len(chain))):
            progressed = False
            for i, (g, total_i, preds) in enumerate(chain):
                if done[i] >= total_i:
                    continue
                if any(done[p] < min(done[i] + off, chain[p][1])
                       for p, off in preds):
                    continue
                next(g)
                done[i] += 1
                progressed = True
            assert progressed, "wavefront gating stuck"

        # ---- per word-chunk: transpose-mode gather of ntaps rows + max tree
        a2_all = const.tile([128, KCH, W], BF16, name="a2_all")
        for wc in range(8):
            tap = gat.tile([128, KCH, ntaps * 128], BF16, tag="tap", name="tap")
            rmax = min(((ntaps * 128 * (wc + 1) + 127) // 128) * 128, T)
            nc.gpsimd.dma_gather(
                out_ap=tap[:],
                in_ap=y1t_d[0:rmax, :],
                idxs_ap=gidx_sb[:, wc * ntaps * 8:(wc + 1) * ntaps * 8],
                num_idxs=ntaps * 128,
                num_idxs_reg=ntaps * 128,
                elem_size=DW,
                transpose=True,
                single_packet=False,
            )
            a2s = a2_all[:, :, wc * 128:(wc + 1) * 128]
            nc.vector.tensor_tensor(
                out=a2s, in0=tap[:, :, 0:128], in1=tap[:, :, 128:256], op=OP.max)
            for j in range(2, ntaps):
                nc.vector.tensor_tensor(
                    out=a2s, in0=a2s, in1=tap[:, :, j * 128:(j + 1) * 128],
                    op=OP.max)

        # ---- proj + bias (ones-row matmul), fp32 out ----
        for wc in range(8):
            po = ps.tile([128, DW], F32, tag="ps", name="po")
            for k in range(KCH):
                nc.tensor.matmul(
                    out=po[:],
                    lhsT=a2_all[:, k, wc * 128:(wc + 1) * 128],
                    rhs=projw_sb[:, k, :],
                    start=(k == 0),
                    stop=False,
                )
            nc.tensor.matmul(
                out=po[:], lhsT=ones_sb[:, 0:128], rhs=projb_sb[:],
                start=False, stop=True,
            )
            ob = obp.tile([128, DW], F32, tag="ob", name="ob")
            nc.vector.tensor_copy(out=ob[:], in_=po[:])
            nc.sync.dma_start(out=out_d[wc * 128:(wc + 1) * 128, :], in_=ob[:])

    nc.compile()
    return nc


@functools.lru_cache(maxsize=2)
def _program(ntaps: int) -> bass.Bass:
    return build_program(ntaps)


def _pack_idx(lin: np.ndarray) -> np.ndarray:
    """SWDGE idx layout: [128, N/16] int16, value n at [p, s] with
    n = s*16 + p%16, replicated across the eight 16-partition groups."""
    n = len(lin)
    assert n % 16 == 0
    arr = np.asarray(lin, dtype=np.int16).reshape(n // 16, 16).T  # [16, n/16]
    return np.tile(arr, (8, 1)).copy()


def _hilo(w, scale):
    """W*scale ~ A + B with both e4m3; returns (A, B) as float32."""
    a = np.asarray(w * scale, dtype=e4_np)
    b = np.asarray(w * scale - a.astype(np.float32), dtype=e4_np)
    return a, b


def prepare(inputs):
    f32 = np.float32
    bt = np.asarray(inputs["byte_tokens"]).astype(np.int64)
    bpe = np.asarray(inputs["bpe_mask"]).astype(np.int64)
    wrd = np.asarray(inputs["word_mask"]).astype(np.int64)
    seg = np.asarray(inputs["seg_ids"]).astype(np.int64)
    emb = np.asarray(inputs["tok_emb"], dtype=f32)
    conv0_w = np.asarray(inputs["conv0_w"], dtype=f32)
    conv0_b = np.asarray(inputs["conv0_b"], dtype=f32)
    conv1_w = np.asarray(inputs["conv1_w"], dtype=f32)
    conv1_b = np.asarray(inputs["conv1_b"], dtype=f32)
    hw_w = {
        (0, "g"): np.asarray(inputs["hw0_wg"], dtype=f32),
        (0, "h"): np.asarray(inputs["hw0_wh"], dtype=f32),
        (1, "g"): np.asarray(inputs["hw1_wg"], dtype=f32),
        (1, "h"): np.asarray(inputs["hw1_wh"], dtype=f32),
    }
    hw_b = {
        (0, "g"): np.asarray(inputs["hw0_bg"], dtype=f32),
        (0, "h"): np.asarray(inputs["hw0_bh"], dtype=f32),
        (1, "g"): np.asarray(inputs["hw1_bg"], dtype=f32),
        (1, "h"): np.asarray(inputs["hw1_bh"], dtype=f32),
    }
    proj_w = np.asarray(inputs["proj_w"], dtype=f32)
    proj_b = np.asarray(inputs["proj_b"], dtype=f32)

    def as_bf16(x):
        return np.ascontiguousarray(x.astype(bf16_np))

    def as_e4(x):
        return np.ascontiguousarray(x.astype(e4_np))

    # combined embedding table: row v + 264*(b + 2*w) = E[v] + b*E4 + w*E3
    # cols 64:128 duplicate 0:64 (for conv0 tap packing via xg2)
    embc = np.zeros((CVOCAB, 128), f32)
    for bm in range(2):
        for wm in range(2):
            r0 = VOCAB * (bm + 2 * wm)
            e = emb + bm * emb[BPE_MARK] + wm * emb[WORD_MARK]
            embc[r0:r0 + VOCAB, :DB] = e
            embc[r0:r0 + VOCAB, DB:] = e

    # conv0: taps 0,1 K-stacked into [128, DW]; tap2 separate
    w0p = np.concatenate([conv0_w[0], conv0_w[1]], axis=0)  # [128, DW]

    # gate weights, plain e4: ag[p, block, l, k, :]
    ag = np.zeros((128, 2, NH, KCH, DW), f32)
    for blk in range(2):
        for l in range(NH):
            wq = np.asarray(hw_w[(blk, "g")][l] * WS, dtype=e4_np).astype(f32)
            ag[:, blk, l] = np.transpose(
                wq.reshape(KCH, 128, DW), (1, 0, 2))

    # h weights, hi/lo: wh[p, block, l, {B,A}, k, :]
    whba = np.zeros((128, 2, NH, 2, KCH, DW), f32)
    for blk in range(2):
        for l in range(NH):
            a, b = _hilo(hw_w[(blk, "h")][l], WS)
            whba[:, blk, l, 0] = np.transpose(
                b.astype(f32).reshape(KCH, 128, DW), (1, 0, 2))
            whba[:, blk, l, 1] = np.transpose(
                a.astype(f32).reshape(KCH, 128, DW), (1, 0, 2))

    # conv1 with residual folded into center tap; hi/lo per tap
    c1w = conv1_w.copy()
    c1w[1] += np.eye(DW, dtype=f32)
    c1ba = np.zeros((128, 2, 3, KCH, DW), f32)
    for t in range(3):
        a, b = _hilo(c1w[t], WS)
        c1ba[:, 0, t] = np.transpose(
            b.astype(f32).reshape(KCH, 128, DW), (1, 0, 2))
        c1ba[:, 1, t] = np.transpose(
            a.astype(f32).reshape(KCH, 128, DW), (1, 0, 2))

    shared = {
        "emb_comb": as_bf16(embc),
        "w0p": as_bf16(w0p),
        "w0t2": as_bf16(conv0_w[2]),
        "ag": as_e4(ag),
        "wh": as_e4(whba),
        "c1": as_e4(c1ba),
        "projw": as_bf16(np.transpose(proj_w.reshape(KCH, 128, DW), (1, 0, 2))),
        "projb": as_bf16(proj_b.reshape(1, DW)),
        "ident": np.eye(128, dtype=bf16_np),
    }

    bias_h = np.zeros((128, 48), f32)
    bias_h[:, COL_C0:COL_C0 + 4] = conv0_b.reshape(KCH, 128).T
    bias_h[:, COL_C0X:COL_C0X + 4] = (conv0_b * XS).reshape(KCH, 128).T
    bias_h[:, COL_C1:COL_C1 + 4] = conv1_b.reshape(KCH, 128).T
    bias_h[:, COL_C1X:COL_C1X + 4] = (conv1_b * XS).reshape(KCH, 128).T
    for blk in (0, 1):
        for l in range(NH):
            bias_h[:, _col_g(blk, l):_col_g(blk, l) + 4] = \
                hw_b[(blk, "g")][l].reshape(KCH, 128).T
            bias_h[:, _col_h(blk, l):_col_h(blk, l) + 4] = \
                hw_b[(blk, "h")][l].reshape(KCH, 128).T
    shared["biases"] = np.ascontiguousarray(bias_h)

    # per-core seg prep; ntaps = max segment length over the whole batch
    counts = np.zeros((B, W), np.int64)
    for b in range(B):
        counts[b] = np.bincount(seg[b], minlength=W)[:W]
    assert (counts >= 1).all(), "empty segments unsupported"
    ntaps = max(int(counts.max()), 2)
    starts = np.zeros((B, W), np.int64)
    starts[:, 1:] = np.cumsum(counts, axis=1)[:, :-1]
    ends = starts + counts - 1

    in_maps = []
    for b in range(B):
        cidx = bt[b] + VOCAB * (bpe[b] + 2 * wrd[b])
        gl = np.empty(8 * ntaps * 128, np.int64)
        for wc in range(8):
            nvec = np.arange(ntaps * 128)
            wv = wc * 128 + (nvec % 128)
            jv = nvec // 128
            gl[wc * ntaps * 128:(wc + 1) * ntaps * 128] = np.minimum(
                starts[b, wv] + jv, ends[b, wv]
            )
        m = dict(shared)
        m["tok_idx"] = _pack_idx(cidx)
        m["gidx"] = np.concatenate(
            [_pack_idx(gl[wc * ntaps * 128:(wc + 1) * ntaps * 128])
             for wc in range(8)], axis=1
        ).copy()
        in_maps.append(m)
    return ntaps, in_maps


def _run(inputs, trace=False, **kwargs):
    ntaps, in_maps = prepare(inputs)
    nc = _program(ntaps)
    res = run_bass_kernel_spmd(
        nc, in_maps, core_ids=list(range(NCORES)), trace=trace, **kwargs
    )
    out = np.stack([res.results[b]["out"] for b in range(B)], axis=0)
    return out.astype(np.float32), res


def kernel(**inputs) -> np.ndarray:
    out, _ = _run(inputs, trace=False)
    return out


def run_traced(inputs, **kwargs):
    return _run(inputs, trace=True, **kwargs)


# revision 63
# speedup vs baseline: 1.0181x; 1.0181x over previous
"""Trainium2 Bass kernel for nn_ByteSequenceEmbedder (fp8 DoubleRow version).

Model (per sequence, 8 sequences data-parallel over 8 NeuronCores):
  x  = tok_emb[tokens] + bpe*E[4] + word*E[3]                 [T=4096, 64]
  x  = relu(conv3(x, W0) + b0); 2x highway(512)               [T, 512]
  x  = relu(conv3(x, W1) + b1 + x); 2x highway(512)           [T, 512]
  x  = per-word segment max (ragged, sorted seg_ids, W=1024)  [W, 512]
  out= x @ Pw + Pb                                            [W, 512]

Precision strategy (validated in numpy sim, rel_err 4.5e-3 == bf16 level):
 - highway gate gemms: plain fp8e4 DoubleRow (sigmoid attenuates noise)
 - highway h gemms + conv1: hi/lo-split fp8e4 DoubleRow:
     W*WS ~ A + B (A=e4(W*WS), B=e4(W*WS-A)); y*XS ~ u + v
     W@y * WS*XS ~ A@u + A@v + B@u  (3 products, B@v dropped)
   DoubleRow packs 2 (lhsT,rhs) K<=128 tile pairs per instr at 0.5
   cyc/row -> 4x bf16 throughput per product.
 - conv0 (K=64): bf16 with taps 0+1 K-packed into one K=128 matmul
 - proj, transposes: bf16. Element-wise stream (y, g, h) stays bf16.
 - XS=128, WS=64 (powers of 2; PSUM evicted with scale 2^-13)

Engine split: ACT: psum evictions (g sigmoid / h relu / conv relu, fp8 u
from conv psums, fp8 u' copies of highway outputs). DVE: highway combine
(3x tensor_tensor bf16), all v-residual stt ops, transpose-psum/proj-psum
evacuation copies, segment-max tree. Pool: SWDGE gathers only (walrus
rejects ALU ops on Pool). PE: matmuls + transposes. PSUM: 4x[128,1024]
slots; u'/v' emission staggered one superchunk behind the evictions so the
in-order ACT/DVE queues never stall the next layer's PE stream.
"""

import functools
import os
import sys

import numpy as np

for _p in ("/opt/trn_rl_repo", "/root/.axon_site/_ro/trn_rl_repo"):
    if os.path.isdir(_p) and _p not in sys.path:
        sys.path.append(_p)

import ml_dtypes  # noqa: E402

from contextlib import ExitStack  # noqa: E402

from concourse import bacc, bass, mybir, tile  # noqa: E402
from concourse import library_config  # noqa: E402
from concourse.bass_utils import run_bass_kernel_spmd  # noqa: E402

B, T, W = 8, 4096, 1024
DB, DW = 64, 512
NH = 2
VOCAB = 264
BPE_MARK, WORD_MARK = 4, 3
SC = 1024          # tokens per super-chunk (psum tile free size)
NSC = T // SC
NMM = 512          # bf16 matmul moving columns
NDR = 256          # DoubleRow matmul output columns (rhs moving = 512)
MCH = DW // 128    # output-feature chunks
KCH = DW // 128    # contraction chunks
NCORES = 8
CVOCAB = 4 * VOCAB  # combined (tok, bpe, word) vocabulary

XS = 128.0          # activation fp8 scale (2^7)
WS = 64.0           # weight fp8 scale (2^6)
EVS = 1.0 / (XS * WS)   # psum eviction scale 2^-13

BF16 = mybir.dt.bfloat16
F32 = mybir.dt.float32
E4 = mybir.dt.float8e4
I16 = mybir.dt.int16
AF = mybir.ActivationFunctionType
OP = mybir.AluOpType
DR = mybir.MatmulPerfMode.DoubleRow

bf16_np = ml_dtypes.bfloat16
e4_np = ml_dtypes.float8_e4m3

# bias column layout in bias_sb [128, 48] f32
COL_C0, COL_C0X = 0, 4        # conv0_b, conv0_b*XS
COL_C1, COL_C1X = 8, 12       # conv1_b, conv1_b*XS


def _col_g(block, l):
    return 16 + (block * NH + l) * 8


def _col_h(block, l):
    return 16 + (block * NH + l) * 8 + 4


def build_program(ntaps: int) -> bass.Bass:
    nc = bacc.Bacc("TRN2", target_bir_lowering=False, debug=False)

    def din(name, shape, dtype):
        return nc.dram_tensor(name, list(shape), dtype, kind="ExternalInput")

    emb_d = din("emb_comb", (CVOCAB, 128), BF16)
    tokidx_d = din("tok_idx", (128, T // 16), I16)
    w0p_d = din("w0p", (128, DW), BF16)          # taps 0,1 K-stacked
    w0t2_d = din("w0t2", (DB, DW), BF16)         # tap 2
    ag_d = din("ag", (128, 2, NH, KCH, DW), E4)  # plain gate weights [block]
    wh_d = din("wh", (128, 2, NH, 2, KCH, DW), E4)  # h weights [block][B;A]
    c1_d = din("c1", (128, 2, 3, KCH, DW), E4)      # conv1 [B;A][tap]
    projw_d = din("projw", (128, KCH, DW), BF16)
    projb_d = din("projb", (1, DW), BF16)
    bias_d = din("biases", (128, 48), F32)
    ident_d = din("ident", (128, 128), BF16)
    gidx_d = din("gidx", (128, 8 * ntaps * 8), I16)
    out_d = nc.dram_tensor("out", [W, DW], F32, kind="ExternalOutput")
    y1t_d = nc.dram_tensor("y1t", [T, DW], BF16, kind="Internal")

    with tile.TileContext(nc) as tc, ExitStack() as ctx:
        const = ctx.enter_context(tc.tile_pool(name="const", bufs=1))
        ps = ctx.enter_context(tc.tile_pool(name="psp", bufs=4, space="PSUM"))
        gp = ctx.enter_context(tc.tile_pool(name="gpool", bufs=8))
        hp = ctx.enter_context(tc.tile_pool(name="hpool", bufs=4))
        dp = ctx.enter_context(tc.tile_pool(name="dpool", bufs=3))
        uvp = ctx.enter_context(tc.tile_pool(name="uvpool", bufs=2))
        tp = ctx.enter_context(tc.tile_pool(name="tpool", bufs=4))
        gat = ctx.enter_context(tc.tile_pool(name="gat", bufs=2))
        obp = ctx.enter_context(tc.tile_pool(name="obp", bufs=2))

        nc.gpsimd.load_library(library_config.mlp)

        def load(dram_t, shape, dtype, name):
            t = const.tile(shape, dtype, name=name)
            nc.sync.dma_start(out=t[:], in_=dram_t[:])
            return t

        # conv0 dependencies first (HWDGE is FIFO per engine)
        tokidx_sb = load(tokidx_d, [128, T // 16], I16, "tokidx_sb")
        w0p_sb = load(w0p_d, [128, DW], BF16, "w0p_sb")
        w0t2_sb = load(w0t2_d, [DB, DW], BF16, "w0t2_sb")
        bias_sb = load(bias_d, [128, 48], F32, "bias_sb")

        # ---- embedding gather: xg[p, t] = emb_comb[cidx[t], p] ----
        # table cols 64:128 duplicate 0:64 so xg[64:128,:] == x too.
        # Shares the gather pool: dead after conv0.
        xg = gat.tile([128, T], BF16, tag="tap", name="xg")
        EC = T // 4
        for r in range(4):
            nc.gpsimd.dma_gather(
                out_ap=xg[:, r * EC:(r + 1) * EC].rearrange(
                    "p (c n) -> p c n", c=1),
                in_ap=emb_d[:],
                idxs_ap=tokidx_sb[:, r * (EC // 16):(r + 1) * (EC // 16)],
                num_idxs=EC,
                num_idxs_reg=EC,
                elem_size=128,
                transpose=True,
                single_packet=False,
            )

        ag_sb = load(ag_d, [128, 2, NH, KCH, DW], E4, "ag_sb")
        wh_sb = load(wh_d, [128, 2, NH, 2, KCH, DW], E4, "wh_sb")
        c1_sb = load(c1_d, [128, 2, 3, KCH, DW], E4, "c1_sb")
        projw_sb = load(projw_d, [128, KCH, DW], BF16, "projw_sb")
        projb_sb = load(projb_d, [1, DW], BF16, "projb_sb")
        gidx_sb = load(gidx_d, [128, 8 * ntaps * 8], I16, "gidx_sb")
        ident_sb = load(ident_d, [128, 128], BF16, "ident_sb")
        ones_sb = const.tile([1, 128], BF16, name="ones_sb")
        nc.vector.memset(ones_sb[:], 1.0)

        # xg2: partitions 0:64 = x[t-1], 64:128 = x[t]  (conv0 tap packing)
        # Shares the uv pool: dead before uv1 is written.
        xg2 = uvp.tile([128, T], BF16, tag="uv", name="xg2")
        nc.vector.memset(xg2[0:DB, 0:1], 0.0)
        for r in range(4):
            c0, c1 = r * EC, (r + 1) * EC
            nc.vector.tensor_copy(
                out=xg2[0:DB, c0 + 1:c1 + 1] if r < 3
                else xg2[0:DB, c0 + 1:T],
                in_=xg[0:DB, c0:c1] if r < 3 else xg[0:DB, c0:T - 1])
            nc.vector.tensor_copy(out=xg2[DB:128, c0:c1], in_=xg[DB:128, c0:c1])

        y_tiles = [const.tile([128, T], BF16, name=f"y_{m}")
                   for m in range(MCH)]

        def uv_tile(name):
            return uvp.tile([128, 2 * KCH, T], E4, tag="uv", name=name)

        def conv_v_pass(uv_out, sc):
            for m in range(MCH):
                cols = slice(sc * SC, sc * SC + SC)
                nc.vector.scalar_tensor_tensor(
                    out=uv_out[:, KCH + m, cols], in0=y_tiles[m][:, cols],
                    scalar=XS, in1=uv_out[:, m, cols],
                    op0=OP.mult, op1=OP.subtract,
                )

        # ---------- conv0 (bf16, 2-tap packed), evict y0 + u0 ----------
        uv0 = uv_tile("uv0")

        def conv0_gen():
            for sc in range(NSC):
                base = sc * SC
                for m in range(MCH):
                    mc = slice(m * 128, (m + 1) * 128)
                    pc = ps.tile([128, SC], F32, tag="ps", name="pc")
                    for n in range(SC // NMM):
                        c0 = n * NMM
                        t0 = base + c0
                        # taps 0+1 via xg2 (K=128), full region
                        nc.tensor.matmul(
                            out=pc[:, c0:c0 + NMM],
                            lhsT=w0p_sb[:, mc],
                            rhs=xg2[:, t0:t0 + NMM],
                            start=True, stop=False,
                        )
                        # tap 2 from xg[0:64] shifted +1, clipped at T
                        lo = t0 + 1
                        ln = min(NMM, T - lo)
                        nc.tensor.matmul(
                            out=pc[:, c0:c0 + ln],
                            lhsT=w0t2_sb[:, mc],
                            rhs=xg[0:DB, lo:lo + ln],
                            start=False, stop=True,
                        )
                    cols = slice(base, base + SC)
                    nc.scalar.activation(
                        out=y_tiles[m][:, cols], in_=pc[:], func=AF.Relu,
                        bias=bias_sb[:, COL_C0 + m:COL_C0 + m + 1],
                    )
                    nc.scalar.activation(
                        out=uv0[:, m, cols], in_=pc[:], func=AF.Relu,
                        bias=bias_sb[:, COL_C0X + m:COL_C0X + m + 1],
                        scale=XS,
                    )
                if sc >= 1:
                    conv_v_pass(uv0, sc - 1)
                yield
            conv_v_pass(uv0, NSC - 1)
            yield

        # ---------- highway layer helper ----------
        def uv_pass(uv_out, sc):
            """fp8 hi/lo of y for superchunk sc: u on ACT (except one
            feature chunk on DVE for engine balance), v on DVE."""
            for m in range(MCH):
                cols = slice(sc * SC, sc * SC + SC)
                yap = y_tiles[m][:, cols]
                if m == MCH - 1:
                    nc.vector.tensor_scalar(
                        out=uv_out[:, m, cols], in0=yap,
                        scalar1=XS, scalar2=None, op0=OP.mult,
                    )
                else:
                    nc.scalar.activation(
                        out=uv_out[:, m, cols], in_=yap,
                        func=AF.Copy, scale=XS,
                    )
                nc.vector.scalar_tensor_tensor(
                    out=uv_out[:, KCH + m, cols], in0=yap,
                    scalar=XS, in1=uv_out[:, m, cols],
                    op0=OP.mult, op1=OP.subtract,
                )

        def hw_layer(block, l, uv_in, uv_out, last, tail_fn=None):
            """One highway layer: consume uv_in, update y_tiles in place,
            and (unless last) produce uv_out = fp8 hi/lo of the new y.

            Emission order staggers the two superchunks so that sc0's
            u/v (ACT+DVE) are queued behind sc1's g evictions: ACT keeps
            working while DVE finishes sc0 combines, and the next layer's
            PE work only waits on sc0's u/v."""
            colg, colh = _col_g(block, l), _col_h(block, l)

            def g_phase(sc):
                base = sc * SC
                g_tiles = []
                for m in range(MCH):
                    mc = slice(m * 128, (m + 1) * 128)
                    pg = ps.tile([128, SC], F32, tag="ps", name="pg")
                    for n in range(SC // NDR):
                        c0 = n * NDR
                        for kp in range(KCH // 2):
                            nc.tensor.matmul(
                                out=pg[:, c0:c0 + NDR],
                                lhsT=ag_sb[:, block, l, 2 * kp:2 * kp + 2, mc],
                                rhs=uv_in[:, 2 * kp:2 * kp + 2,
                                          base + c0:base + c0 + NDR],
                                start=(kp == 0), stop=(kp == KCH // 2 - 1),
                                perf_mode=DR,
                            )
                    g = gp.tile([128, SC], BF16, tag="g", name="g")
                    nc.scalar.activation(
                        out=g[:], in_=pg[:], func=AF.Sigmoid,
                        bias=bias_sb[:, colg + m:colg + m + 1], scale=EVS,
                    )
                    g_tiles.append(g)
                return g_tiles

            def h_combine_phase(sc, g_tiles):
                base = sc * SC
                for m in range(MCH):
                    mc = slice(m * 128, (m + 1) * 128)
                    ph = ps.tile([128, SC], F32, tag="ps", name="ph")
                    for n in range(SC // NDR):
                        c0, c1 = base + n * NDR, base + (n + 1) * NDR
                        for kp in range(KCH // 2):  # main: A@u chunk pairs
                            nc.tensor.matmul(
                                out=ph[:, n * NDR:(n + 1) * NDR],
                                lhsT=wh_sb[:, block, l, 1,
                                           2 * kp:2 * kp + 2, mc],
                                rhs=uv_in[:, 2 * kp:2 * kp + 2, c0:c1],
                                start=(kp == 0), stop=False,
                                perf_mode=DR,
                            )
                        for k in range(KCH):        # cross: B@u + A@v
                            nc.tensor.matmul(
                                out=ph[:, n * NDR:(n + 1) * NDR],
                                lhsT=wh_sb[:, block, l, :, k, mc],
                                rhs=uv_in[:, k:k + KCH + 1:KCH, c0:c1],
                                start=False, stop=(k == KCH - 1),
                                perf_mode=DR,
                            )
                    h = hp.tile([128, SC], BF16, tag="h", name="h")
                    nc.scalar.activation(
                        out=h[:], in_=ph[:], func=AF.Relu,
                        bias=bias_sb[:, colh + m:colh + m + 1], scale=EVS,
                    )
                    cols = slice(base, base + SC)
                    yap = y_tiles[m][:, cols]
                    d = dp.tile([128, SC], BF16, tag="d", name="d")
                    nc.vector.tensor_tensor(
                        out=d[:], in0=h[:], in1=yap, op=OP.subtract)
                    nc.vector.tensor_tensor(
                        out=d[:], in0=g_tiles[m][:], in1=d[:], op=OP.mult)
                    nc.vector.tensor_tensor(
                        out=yap, in0=yap, in1=d[:], op=OP.add)

            gs = g_phase(0)
            h_combine_phase(0, gs)
            yield
            for sc in range(1, NSC):
                gs = g_phase(sc)
                if not last:
                    uv_pass(uv_out, sc - 1)
                h_combine_phase(sc, gs)
                if tail_fn is not None:
                    tail_fn(sc - 1)
                yield
            if not last:
                uv_pass(uv_out, NSC - 1)
            if tail_fn is not None:
                tail_fn(NSC - 1)
            yield

        # ---------- conv1 (hi/lo fp8 DR, residual folded), evict y1+u1 ----
        uv1 = uv_tile("uv1")
        uv2 = uv_tile("uv2")
        uv3 = uv_tile("uv3")
        uv4 = uv_tile("uv4")

        def conv1_gen():
            for sc in range(NSC):
                base = sc * SC
                for m in range(MCH):
                    mc = slice(m * 128, (m + 1) * 128)
                    pc = ps.tile([128, SC], F32, tag="ps", name="pc1")
                    for n in range(SC // NDR):
                        c0 = n * NDR
                        first = True
                        for t in (1, 0, 2):
                            lo = base + c0 + (t - 1)
                            ln = NDR
                            o0 = c0
                            if lo < 0:
                                lo, ln, o0 = 0, NDR - 1, c0 + 1
                            elif lo + ln > T:
                                ln = T - lo
                            for kp in range(KCH // 2):  # main A@u
                                nc.tensor.matmul(
                                    out=pc[:, o0:o0 + ln],
                                    lhsT=c1_sb[:, 1, t,
                                               2 * kp:2 * kp + 2, mc],
                                    rhs=uv2[:, 2 * kp:2 * kp + 2,
                                            lo:lo + ln],
                                    start=first, stop=False,
                                    perf_mode=DR,
                                )
                                first = False
                            for k in range(KCH):        # cross B@u + A@v
                                nc.tensor.matmul(
                                    out=pc[:, o0:o0 + ln],
                                    lhsT=c1_sb[:, :, t, k, mc],
                                    rhs=uv2[:, k:k + KCH + 1:KCH,
                                            lo:lo + ln],
                                    start=False,
                                    stop=(t == 2 and k == KCH - 1),
                                    perf_mode=DR,
                                )
                    cols = slice(base, base + SC)
                    nc.scalar.activation(
                        out=y_tiles[m][:, cols], in_=pc[:], func=AF.Relu,
                        bias=bias_sb[:, COL_C1 + m:COL_C1 + m + 1],
                        scale=EVS,
                    )
                    nc.scalar.activation(
                        out=uv3[:, m, cols], in_=pc[:], func=AF.Relu,
                        bias=bias_sb[:, COL_C1X + m:COL_C1X + m + 1],
                        scale=EVS * XS,
                    )
                if sc >= 1:
                    conv_v_pass(uv3, sc - 1)
                yield
            conv_v_pass(uv3, NSC - 1)
            yield
        # ---- transpose y -> token-major, bounce to DRAM ----
        # batched: 8 transposes -> one [128,1024] psum tile -> one DVE copy
        # -> one 256-row DMA; interleaved per superchunk into the last
        # highway layer so y1t is mostly written (and gathers fire) before
        # the body ends.
        def transpose_chunk(sc):
            for gi in range(sc * (SC // 256), (sc + 1) * (SC // 256)):
                pt = ps.tile([128, 1024], BF16, tag="ps", name="pt")
                for i in range(2):
                    col = gi * 256 + i * 128
                    for m in range(MCH):
                        nc.tensor.transpose(
                            out=pt[:, i * 512 + m * 128:
                                   i * 512 + (m + 1) * 128],
                            in_=y_tiles[m][:, col:col + 128],
                            identity=ident_sb[:],
                        )
                st = tp.tile([128, 1024], BF16, tag="y1t", name="st")
                nc.vector.tensor_copy(out=st[:], in_=pt[:])
                row0 = gi * 256
                nc.sync.dma_start(
                    out=y1t_d[row0:row0 + 256, :].rearrange(
                        "(i p) f -> p i f", i=2),
                    in_=st[:].rearrange("p (i f) -> p i f", i=2),
                )

        # ---------- wavefront driver ----------
        # Each phase is a generator with NSC+1 steps. A consumer phase's
        # step s needs its producer's uv slice for superchunk s, which the
        # producer finishes `off` steps later (staggered v/u passes; conv1
        # additionally needs one halo column of the next superchunk).
        # Interleaved emission puts ACT/DVE work of highway layers inside
        # conv1's PE-heavy window and vice versa.
        # Highway layers are split into compute (matmuls+evictions+combine)
        # and uvout (fp8 u/v of the updated y) generators so compute can
        # overlap a predecessor at depth 2 while uv WRITES wait until the
        # recycled uv buffer's previous readers have been emitted (the uv
        # pool double-buffers; writer-before-reader emission wedges the
        # in-order ACT queue into a deadlock). Gate: consumer step s waits
        # until done[pred] >= min(s + off, total[pred]).
        chain = [
            (conv0_gen(), 5, []),                                  # 0
            (hw_layer(0, 0, uv0, uv1, False), 5, [(0, 3)]),        # 1
            (hw_layer(0, 1, uv1, uv2, False), 5, [(1, 3)]),        # 2
            (conv1_gen(), 5, [(2, 4)]),                            # 3
            (hw_layer(1, 0, uv3, uv4, False), 5, [(3, 3)]),        # 4
            (hw_layer(1, 1, uv4, None, True,
                      tail_fn=transpose_chunk), 5, [(4, 2)]),      # 5
        ]
        done = [0] * len(chain)
        while any(done[i] < chain[i][1] for i in range(len(chain))):
            progressed = False
            for i, (g, total_i, preds) in enumerate(chain):
                if done[i] >= total_i:
                    continue
                if any(done[p] < min(done[i] + off, chain[p][1])
                       for p, off in preds):
                    continue
                next(g)
                done[i] += 1
                progressed = True
            assert progressed, "wavefront gating stuck"

        # ---- per word-chunk: transpose-mode gather of ntaps rows + max tree
        a2_all = const.tile([128, KCH, W], BF16, name="a2_all")
        for wc in range(8):
            tap = gat.tile([128, KCH, ntaps * 128], BF16, tag="tap", name="tap")
            rmax = min(((ntaps * 128 * (wc + 1) + 127) // 128) * 128, T)
            nc.gpsimd.dma_gather(
                out_ap=tap[:],
                in_ap=y1t_d[0:rmax, :],
                idxs_ap=gidx_sb[:, wc * ntaps * 8:(wc + 1) * ntaps * 8],
                num_idxs=ntaps * 128,
                num_idxs_reg=ntaps * 128,
                elem_size=DW,
                transpose=True,
                single_packet=False,
            )
            a2s = a2_all[:, :, wc * 128:(wc + 1) * 128]
            nc.vector.tensor_tensor(
                out=a2s, in0=tap[:, :, 0:128], in1=tap[:, :, 128:256], op=OP.max)
            for j in range(2, ntaps):
                nc.vector.tensor_tensor(
                    out=a2s, in0=a2s, in1=tap[:, :, j * 128:(j + 1) * 128],
                    op=OP.max)

        # ---- proj + bias (ones-row matmul), fp32 out ----
        for wc in range(8):
            po = ps.tile([128, DW], F32, tag="ps", name="po")
            for k in range(KCH):
                nc.tensor.matmul(
                    out=po[:],
                    lhsT=a2_all[:, k, wc * 128:(wc + 1) * 128],
                    rhs=projw_sb[:, k, :],
                    start=(k == 0),
                    stop=False,
                )
            nc.tensor.matmul(
                out=po[:], lhsT=ones_sb[:, 0:128], rhs=projb_sb[:],
                start=False, stop=True,
            )
            ob = obp.tile([128, DW], F32, tag="ob", name="ob")
            nc.vector.tensor_copy(out=ob[:], in_=po[:])
            nc.sync.dma_start(out=out_d[wc * 128:(wc + 1) * 128, :], in_=ob[:])

    nc.compile()
    return nc


@functools.lru_cache(maxsize=2)
def _program(ntaps: int) -> bass.Bass:
    return build_program(ntaps)


def _pack_idx(lin: np.ndarray) -> np.ndarray:
    """SWDGE idx layout: [128, N/16] int16, value n at [p, s] with
    n = s*16 + p%16, replicated across the eight 16-partition groups."""
    n = len(lin)
    assert n % 16 == 0
    arr = np.asarray(lin, dtype=np.int16).reshape(n // 16, 16).T  # [16, n/16]
    return np.tile(arr, (8, 1)).copy()


def _hilo(w, scale):
    """W*scale ~ A + B with both e4m3; returns (A, B) as float32."""
    a = np.asarray(w * scale, dtype=e4_np)
    b = np.asarray(w * scale - a.astype(np.float32), dtype=e4_np)
    return a, b


def prepare(inputs):
    f32 = np.float32
    bt = np.asarray(inputs["byte_tokens"]).astype(np.int64)
    bpe = np.asarray(inputs["bpe_mask"]).astype(np.int64)
    wrd = np.asarray(inputs["word_mask"]).astype(np.int64)
    seg = np.asarray(inputs["seg_ids"]).astype(np.int64)
    emb = np.asarray(inputs["tok_emb"], dtype=f32)
    conv0_w = np.asarray(inputs["conv0_w"], dtype=f32)
    conv0_b = np.asarray(inputs["conv0_b"], dtype=f32)
    conv1_w = np.asarray(inputs["conv1_w"], dtype=f32)
    conv1_b = np.asarray(inputs["conv1_b"], dtype=f32)
    hw_w = {
        (0, "g"): np.asarray(inputs["hw0_wg"], dtype=f32),
        (0, "h"): np.asarray(inputs["hw0_wh"], dtype=f32),
        (1, "g"): np.asarray(inputs["hw1_wg"], dtype=f32),
        (1, "h"): np.asarray(inputs["hw1_wh"], dtype=f32),
    }
    hw_b = {
        (0, "g"): np.asarray(inputs["hw0_bg"], dtype=f32),
        (0, "h"): np.asarray(inputs["hw0_bh"], dtype=f32),
        (1, "g"): np.asarray(inputs["hw1_bg"], dtype=f32),
        (1, "h"): np.asarray(inputs["hw1_bh"], dtype=f32),
    }
    proj_w = np.asarray(inputs["proj_w"], dtype=f32)
    proj_b = np.asarray(inputs["proj_b"], dtype=f32)

    def as_bf16(x):
        return np.ascontiguousarray(x.astype(bf16_np))

    def as_e4(x):
        return np.ascontiguousarray(x.astype(e4_np))

    # combined embedding table: row v + 264*(b + 2*w) = E[v] + b*E4 + w*E3
    # cols 64:128 duplicate 0:64 (for conv0 tap packing via xg2)
    embc = np.zeros((CVOCAB, 128), f32)
    for bm in range(2):
        for wm in range(2):
            r0 = VOCAB * (bm + 2 * wm)
            e = emb + bm * emb[BPE_MARK] + wm * emb[WORD_MARK]
            embc[r0:r0 + VOCAB, :DB] = e
            embc[r0:r0 + VOCAB, DB:] = e

    # conv0: taps 0,1 K-stacked into [128, DW]; tap2 separate
    w0p = np.concatenate([conv0_w[0], conv0_w[1]], axis=0)  # [128, DW]

    # gate weights, plain e4: ag[p, block, l, k, :]
    ag = np.zeros((128, 2, NH, KCH, DW), f32)
    for blk in range(2):
        for l in range(NH):
            wq = np.asarray(hw_w[(blk, "g")][l] * WS, dtype=e4_np).astype(f32)
            ag[:, blk, l] = np.transpose(
                wq.reshape(KCH, 128, DW), (1, 0, 2))

    # h weights, hi/lo: wh[p, block, l, {B,A}, k, :]
    whba = np.zeros((128, 2, NH, 2, KCH, DW), f32)
    for blk in range(2):
        for l in range(NH):
            a, b = _hilo(hw_w[(blk, "h")][l], WS)
            whba[:, blk, l, 0] = np.transpose(
                b.astype(f32).reshape(KCH, 128, DW), (1, 0, 2))
            whba[:, blk, l, 1] = np.transpose(
                a.astype(f32).reshape(KCH, 128, DW), (1, 0, 2))

    # conv1 with residual folded into center tap; hi/lo per tap
    c1w = conv1_w.copy()
    c1w[1] += np.eye(DW, dtype=f32)
    c1ba = np.zeros((128, 2, 3, KCH, DW), f32)
    for t in range(3):
        a, b = _hilo(c1w[t], WS)
        c1ba[:, 0, t] = np.transpose(
            b.astype(f32).reshape(KCH, 128, DW), (1, 0, 2))
        c1ba[:, 1, t] = np.transpose(
            a.astype(f32).reshape(KCH, 128, DW), (1, 0, 2))

    shared = {
        "emb_comb": as_bf16(embc),
        "w0p": as_bf16(w0p),
        "w0t2": as_bf16(conv0_w[2]),
        "ag": as_e4(ag),
        "wh": as_e4(whba),
        "c1": as_e4(c1ba),
        "projw": as_bf16(np.transpose(proj_w.reshape(KCH, 128, DW), (1, 0, 2))),
        "projb": as_bf16(proj_b.reshape(1, DW)),
        "ident": np.eye(128, dtype=bf16_np),
    }

    bias_h = np.zeros((128, 48), f32)
    bias_h[:, COL_C0:COL_C0 + 4] = conv0_b.reshape(KCH, 128).T
    bias_h[:, COL_C0X:COL_C0X + 4] = (conv0_b * XS).reshape(KCH, 128).T
    bias_h[:, COL_C1:COL_C1 + 4] = conv1_b.reshape(KCH, 128).T
    bias_h[:, COL_C1X:COL_C1X + 4] = (conv1_b * XS).reshape(KCH, 128).T
    for blk in (0, 1):
        for l in range(NH):
            bias_h[:, _col_g(blk, l):_col_g(blk, l) + 4] = \
                hw_b[(blk, "g")][l].reshape(KCH, 128).T
            bias_h[:, _col_h(blk, l):_col_h(blk, l) + 4] = \
                hw_b[(blk, "h")][l].reshape(KCH, 128).T
    shared["biases"] = np.ascontiguousarray(bias_h)

    # per-core seg prep; ntaps = max segment length over the whole batch
    counts = np.zeros((B, W), np.int64)
    for b in range(B):
        counts[b] = np.bincount(seg[b], minlength=W)[:W]
    assert (counts >= 1).all(), "empty segments unsupported"
    ntaps = max(int(counts.max()), 2)
    starts = np.zeros((B, W), np.int64)
    starts[:, 1:] = np.cumsum(counts, axis=1)[:, :-1]
    ends = starts + counts - 1

    in_maps = []
    for b in range(B):
        cidx = bt[b] + VOCAB * (bpe[b] + 2 * wrd[b])
        gl = np.empty(8 * ntaps * 128, np.int64)
        for wc in range(8):
            nvec = np.arange(ntaps * 128)
            wv = wc * 128 + (nvec % 128)
            jv = nvec // 128
            gl[wc * ntaps * 128:(wc + 1) * ntaps * 128] = np.minimum(
                starts[b, wv] + jv, ends[b, wv]
            )
        m = dict(shared)
        m["tok_idx"] = _pack_idx(cidx)
        m["gidx"] = np.concatenate(
            [_pack_idx(gl[wc * ntaps * 128:(wc + 1) * ntaps * 128])
             for wc in range(8)], axis=1
        ).copy()
        in_maps.append(m)
    return ntaps, in_maps


def _run(inputs, trace=False, **kwargs):
    ntaps, in_maps = prepare(inputs)
    nc = _program(ntaps)
    res = run_bass_kernel_spmd(
        nc, in_maps, core_ids=list(range(NCORES)), trace=trace, **kwargs
    )
    out = np.stack([res.results[b]["out"] for b in range(B)], axis=0)
    return out.astype(np.float32), res


def kernel(**inputs) -> np.ndarray:
    out, _ = _run(inputs, trace=False)
    return out


def run_traced(inputs, **kwargs):
    return _run(inputs, trace=True, **kwargs)
